# revision 16
# baseline (speedup 1.0000x reference)
"""DCT2D kernel for Trainium2 (8 NeuronCores, SPMD data-parallel).

Math: per 8x8 block  out = scale * (C^T (x - 128) C)
  == flat form:  out_flat[n, uv] = sum_xy (x_flat[n, xy] - 128) * T[xy, uv] * s[uv]
  == xc_flat @ W      with W[xy, uv] = T[xy, uv] * s[uv],  xc = x - 128
     (the -128 shift is folded into the host-side pack, so no bias path).

The problem is HBM-bound (50.3 MB/core fp32 I/O @ ~358 GB/s/NC = 141 us
floor), so the kernel narrows the wire dtypes and keeps the math on
device exact enough for the 2e-2 rel-err gate:

  - input:  host packs (x-128) as IN_DT  (fp16: 2.8e-4 rel err;
            int8 round: 3.9e-3)
  - matmul: fp16 weights (blockdiag(W,W) [128,128]), fp16 moving operand,
            fp32 PSUM.  PE streams 1 col/cycle -> ~21 us/pass, never the
            bottleneck.  int8 input is cast int8->fp16 on DVE first
            (matmul has no int8 path).
  - output: OUT_DT.  int8 path folds 1/S_OUT into the weights so the
            PSUM->SBUF move is a pure cast; the HW cast is
            round-to-nearest-even WITH saturation (probed), so clipping
            at 4 sigma = 294 gives the optimal uniform quantizer for the
            ~N(0, 73.6^2) coefficients: ~9.4e-3 rel err total.  Host
            dequantizes by S_OUT.

Device-side layout: the PE contracts over the partition dim, so the host
pre-transposes each core's shard to [128, R/2] -- two consecutive blocks
stacked on partitions -- and weights are blockdiag(W, W).  DRAM tensors
are tile-major [ntiles, 128, tile_f] so each DMA touches one contiguous
HBM extent.  PSUM is consumed in 2048-col mega-tiles (4 banks).  Engine
assignment (all swept on HW): ALL PSUM drains on ACT (~0.85 ns/col from
PSUM; 24 x 1.75 us/pass), the int8->fp16 in-cast on DVE (packed-mode
SBUF copy, ~13 us/pass).  DVE PSUM reads measure ~2.9 us/mega (~1.65x
ACT) and any DVE drain also delays the next tile's in-cast behind it in
the DVE queue, so DVE/ACT drain splits (parity, bank-split,
mixed-consumer) all measured slower (58-80 us vs 43-50 us).  Input DMAs
ride the sync-engine HWDGE ring, output DMAs the scalar-engine ring
(separate FIFOs).  Pure-DMA floor measured 34 us/pass (369 GB/s mixed
read+write); steady state lands at 38-50 us/pass depending on machine
load (3.2-4.2x over the 161 us fp32 baseline).
"""

import sys

if "/opt/trn_rl_repo" not in sys.path:
    sys.path.insert(0, "/opt/trn_rl_repo")

import numpy as np

import concourse.bass as bass  # noqa: F401
import concourse.mybir as mybir
import concourse.tile as tile
from concourse import bacc
from concourse.bass_utils import run_bass_kernel_spmd

N_CORES = 8
BLOCK = 8
B_DIM = 262144
C_DIM = 3
NBLK = B_DIM * C_DIM          # 786432 total 8x8 blocks
R = NBLK // N_CORES           # 98304 blocks per core
RP = R // 2                   # 49152 packed columns per core
TILE_F = 8192                 # columns per SBUF tile
MM_F = 512                    # columns per matmul (one PSUM bank, fp32)
PS_F = 2048                   # columns per PSUM mega-tile (4 banks)

IN_DT = "int8"                # "float16" | "int8"
OUT_DT = "int8"               # "float16" | "int8"
SIGMA = 255.0 / np.sqrt(12.0)         # per-coefficient output stddev
# 4.5 sigma clip minimizes measured rel err (1.06e-2): the coefficient
# distribution has fatter-than-Gaussian 4-5 sigma tails.
S_OUT = np.float32(4.5 * SIGMA / 127.0)  # int8 output dequant scale

_CACHE = {}
last_results = None  # BassKernelResults of the most recent run (for test harness)
last_in_maps = None  # per-core input maps of the most recent run (for bench2)

_DT = {"float16": mybir.dt.float16, "int8": mybir.dt.int8,
       "float32": mybir.dt.float32}
_NP = {"float16": np.float16, "int8": np.int8, "float32": np.float32}


def _emit_pass(nc, pools, w_sb, xt, out_t, rp, tile_f, in_dt, out_dt):
    f32 = mybir.dt.float32
    f16 = mybir.dt.float16
    xpool, fpool, opool, pspool = pools
    for t in range(rp // tile_f):
        xin = xpool.tile([128, tile_f], _DT[in_dt])
        nc.sync.dma_start(xin[:], xt[t])
        if in_dt == "int8":
            src = fpool.tile([128, tile_f], f16)
            nc.vector.tensor_copy(src[:], xin[:])
        else:
            src = xin
        ot = opool.tile([128, tile_f], _DT[out_dt])
        for j in range(tile_f // PS_F):
            ps = pspool.tile([128, PS_F], f32)
            for k in range(PS_F // MM_F):
                col = k * MM_F
                nc.tensor.matmul(
                    ps[:, col : col + MM_F], w_sb[:],
                    src[:, j * PS_F + col : j * PS_F + col + MM_F],
                    start=True, stop=True,
                )
            # All PSUM drains on ACT (sustains ~0.85 ns/col from PSUM and
            # is otherwise idle); DVE carries only the int8->fp16 in-cast.
            # Swept alternatives (DVE/ACT parity split, bank splits,
            # mixed-consumer megas) all measured slower -- the Tile
            # scheduler serializes PSUM tiles with mixed consumers.
            osl = ot[:, j * PS_F : (j + 1) * PS_F]
            nc.scalar.activation(
                osl, ps[:], mybir.ActivationFunctionType.Copy,
                bias=0.0, scale=1.0,
            )
        nc.scalar.dma_start(out_t[t], ot[:])


def _build_nc(rp=RP, tile_f=TILE_F, n_passes=1, in_dt=None, out_dt=None):
    in_dt = in_dt or IN_DT
    out_dt = out_dt or OUT_DT
    f16 = mybir.dt.float16
    nt = rp // tile_f
    nc = bacc.Bacc(None, target_bir_lowering=False, debug=False)
    xt = nc.declare_dram_parameter("xt", [nt, 128, tile_f], _DT[in_dt], isOutput=False)
    w = nc.declare_dram_parameter("w", [128, 128], f16, isOutput=False)
    out = nc.declare_dram_parameter(
        "out", [nt, 128, tile_f], _DT[out_dt], isOutput=True
    )

    with tile.TileContext(nc) as tc:
        with (
            tc.tile_pool(name="consts", bufs=1) as cpool,
            tc.tile_pool(name="xin", bufs=6) as xpool,
            tc.tile_pool(name="xf", bufs=3) as fpool,
            tc.tile_pool(name="out", bufs=6) as opool,
            tc.tile_pool(name="ps", bufs=2, space="PSUM") as pspool,
        ):
            w_sb = cpool.tile([128, 128], f16)
            nc.sync.dma_start(w_sb[:], w[:])
            pools = (xpool, fpool, opool, pspool)
            for _ in range(n_passes):
                _emit_pass(nc, pools, w_sb, xt, out, rp, tile_f, in_dt, out_dt)
    nc.compile()
    return nc


def _consts(dct_tensor, scale, out_dt):
    t_flat = np.asarray(dct_tensor, dtype=np.float64).reshape(64, 64)
    s_flat = np.asarray(scale, dtype=np.float64).reshape(64)
    w64 = t_flat * s_flat[None, :]
    if out_dt == "int8":
        w64 = w64 / float(S_OUT)
    w = np.zeros((128, 128), dtype=np.float16)
    w[:64, :64] = w64.astype(np.float16)
    w[64:, 64:] = w64.astype(np.float16)
    return w


def kernel(x, dct_tensor, scale):
    in_dt, out_dt = IN_DT, OUT_DT
    w = _consts(dct_tensor, scale, out_dt)

    from concurrent.futures import ThreadPoolExecutor

    nt = RP // TILE_F
    xf = np.ascontiguousarray(np.asarray(x, dtype=np.float32)).reshape(NBLK, 64)

    def _pack(c):
        shard = xf[c * R : (c + 1) * R] - 128.0
        if in_dt == "int8":
            shard = np.rint(shard).astype(np.int8)
        else:
            shard = shard.astype(np.float16)
        # xt[t, p*64+k, f] = shard[2*(t*TILE_F+f)+p, k]
        return np.ascontiguousarray(
            shard.reshape(nt, TILE_F, 2, 64).transpose(0, 2, 3, 1)
        ).reshape(nt, 128, TILE_F)

    with ThreadPoolExecutor(N_CORES) as pool:
        packs = list(pool.map(_pack, range(N_CORES)))
    in_maps = [{"xt": p, "w": w} for p in packs]
    global last_in_maps
    last_in_maps = in_maps

    key = ("nc", in_dt, out_dt)
    if key not in _CACHE:
        _CACHE[key] = _build_nc(in_dt=in_dt, out_dt=out_dt)
        _CACHE["nc"] = _CACHE[key]
    res = run_bass_kernel_spmd(_CACHE[key], in_maps, core_ids=list(range(N_CORES)))
    global last_results
    last_results = res

    full = np.empty((NBLK, 64), dtype=np.float32)

    def _unpack(c):
        o = np.asarray(res.results[c]["out"])  # [nt, 128, TILE_F] packed
        o = o.reshape(nt, 2, 64, TILE_F).transpose(0, 3, 1, 2).reshape(R, 64)
        if out_dt == "int8":
            full[c * R : (c + 1) * R] = o.astype(np.float32) * S_OUT
        else:
            full[c * R : (c + 1) * R] = o.astype(np.float32)

    with ThreadPoolExecutor(N_CORES) as pool:
        list(pool.map(_unpack, range(N_CORES)))
    return full.reshape(B_DIM, C_DIM, BLOCK, BLOCK)
